# revision 24
# baseline (speedup 1.0000x reference)
"""Single-head causal attention (B=8, T=2048, C=384, H=64) on 8 NeuronCores.

Data-parallel over batch: core b computes attention for batch element b.
v2 pipeline (all matmuls bf16, fp32 PSUM):
  - host pre-transposes x -> xT [C, T], packs Wqk = [Wq|Wk] per 128-chunk of C
  - QK proj: psum[0:64]=qT, psum[64:128]=kT via packed stationary (12 MMs N=512)
  - vT proj: vT[h, t] via Wv stationary (12 MMs N=512); v[s, h] blocks made by
    XBAR DMA-transpose (SBUF->SBUF, free on PE/DVE)
  - qk replicated to both partition halves (2 SBUF->SBUF DMAs) so scores run
    ROW-TILED: even key-block in PE rows 0-63, odd block in rows 64-127,
    concurrently (contraction is H=64)
  - scores stream into [128,1024] PSUM windows (ring of 2); one ACTIVATE(Exp)
    per window (psum f32 -> sbuf bf16 PT); diagonal blocks masked after exp
  - output transposed: outT[h, t] += v_j[s, 0:65].T @ PT_j[s, t] -- v stationary
    is only 65 cols (cheap LDWEIGHTS, big N); ones column 64 gives the softmax
    denominator in outT row 64; accumulated in 4 quarter-bank PSUM tiles
  - outT quarters drain to bf16, XBAR-transposed back to [t, 65]; DVE
    reciprocal+scale; DMA out f32
  - outT MMs for window k are emitted at window k+1's flush so the PE queue
    never blocks on ACT; ACT table preloaded via dummy exp at t=0
"""

import math
import os

import numpy as np
import ml_dtypes

import concourse.bass as bass
import concourse.tile as tile
from concourse import bacc, mybir
from concourse.bass import ds, ts
from concourse.bass_utils import run_bass_kernel_spmd

F32 = mybir.dt.float32
BF16 = mybir.dt.bfloat16

B, T, C, H = 8, 2048, 384, 64
P = 128
NT = T // P          # 16 key/query blocks
NCC = C // P         # 3 contraction chunks
WIN = 1024           # score window columns (2 PSUM banks)
SCALE = 1.0 / math.sqrt(float(C))

LAST_RESULT = None
_PROGRAM = None


def _score_chunks():
    """Yield (j, t0, w, fill) for the score chunk stream.

    Strips sequential (j = 0..15), chunks crossing neither a fill-512 (PSUM
    bank) nor a t-512 boundary. The PE row-group for a chunk is
    (fill//512) % 2: same-bank chunks share a row-group (safely serialized),
    adjacent banks alternate row-groups (run concurrently on the two array
    halves). Concurrent matmuls into the same PSUM bank crash the runtime.
    """
    fill = 0
    for j in range(NT):
        t = P * j
        while t < T:
            w = min(512 - fill % 512, 512 - t % 512, T - t)
            yield (j, t, w, fill)
            t += w
            fill += w


def _emit(tc: tile.TileContext, xT_d, wqk_d, wv_d, mask_d, ident_d, out_d,
          ctx, dbg_d=None):
    nc = tc.nc
    Exp = mybir.ActivationFunctionType.Exp

    sb = ctx.enter_context(tc.tile_pool(name="sb", bufs=1))
    ps = ctx.enter_context(tc.tile_pool(name="ps", bufs=1, space="PSUM"))

    # ---- sbuf tiles -------------------------------------------------------
    wqk_sb = sb.tile([P, NCC, P], BF16, tag="wqk")
    wv_sb = sb.tile([P, NCC, H], BF16, tag="wv")
    mask_sb = sb.tile([P, P], BF16, tag="mask")
    xT = [sb.tile([P, T], BF16, tag=f"xT{c}", name=f"xT{c}") for c in range(NCC)]
    qk_nat = sb.tile([P, T], BF16, tag="qk_nat")   # q in rows 0:64, k in 64:128
    qk_swp = sb.tile([P, T], BF16, tag="qk_swp")   # k in rows 0:64, q in 64:128
    vTsb = sb.tile([H, T], BF16, tag="vTsb")
    v_sb = sb.tile([P, NT, H + 1], BF16, tag="v_sb")
    ident = sb.tile([P, P], BF16, tag="ident")
    n_win = (17408 + WIN - 1) // WIN + 1
    pt_all = sb.tile([P, n_win * WIN], BF16, tag="pt_all")
    outd = sb.tile([H + 1, T], BF16, tag="outd")   # transposed out staging
    dum = sb.tile([1, 8], BF16, tag="dum")
    dum2 = sb.tile([1, 8], BF16, tag="dum2")

    # ---- ACT table preload + constant fills -------------------------------
    nc.vector.memset(dum[:], 0.0)
    nc.scalar.activation(dum2[:], dum[:], Exp, scale=SCALE)
    nc.vector.memset(v_sb[:, :, H], 1.0)

    # ---- input DMAs, split across the two DGE queues (sync + scalar) ----
    # sync: wqk + x chunks c=0,1 (t4-major); scalar: x chunks c=2 + wv/mask/
    # ident.  qk_swp replication also goes on scalar (free until exps start).
    nc.sync.dma_start(wqk_sb[:], wqk_d[:])
    for t4 in range(4):
        nc.scalar.dma_start(xT[2][:, ts(t4, 512)], xT_d[2, t4])
        for c in range(2):
            nc.sync.dma_start(xT[c][:, ts(t4, 512)], xT_d[c, t4])
    nc.scalar.dma_start(wv_sb[:], wv_d[:])
    nc.scalar.dma_start(mask_sb[:], mask_d[:])
    nc.scalar.dma_start(ident[:], ident_d[:])

    # ---- projections for one 512-col t-chunk ------------------------------
    def emit_proj(t4):
        w = ps.tile([P, WIN], F32, tag="win", bufs=3, name=f"proj{t4}")
        for c in range(NCC):
            nc.tensor.matmul(
                w[:, 0:512], wqk_sb[:, c, :], xT[c][:, ts(t4, 512)],
                start=(c == 0), stop=(c == NCC - 1),
            )
        for c in range(NCC):
            nc.tensor.matmul(
                w[0:H, 512:1024], wv_sb[:, c, :], xT[c][:, ts(t4, 512)],
                start=(c == 0), stop=(c == NCC - 1),
            )
        nc.vector.tensor_copy(qk_nat[:, ts(t4, 512)], w[:, 0:512])
        nc.vector.tensor_copy(vTsb[:, ts(t4, 512)], w[0:H, 512:1024])
        # replicate to the other partition half (k -> low, q -> high)
        nc.scalar.dma_start(qk_swp[0:H, ts(t4, 512)], qk_nat[H:P, ts(t4, 512)])
        nc.scalar.dma_start(qk_swp[H:P, ts(t4, 512)], qk_nat[0:H, ts(t4, 512)])

    # ---- main loop --------------------------------------------------------
    # score operands by row-group: rows 0:64 = (k from swp, q from nat),
    # rows 64:128 = (k from nat, q from swp); a chunk's row-group is
    # (fill//512) % 2 so same-bank chunks serialize, adjacent banks overlap
    qA, kA = qk_nat[0:H, :], qk_swp[0:H, :]
    qB, kB = qk_swp[H:P, :], qk_nat[H:P, :]

    out_v = out_d.rearrange("(g i p) h -> g p i h", p=P, i=4)

    all_chunks = list(_score_chunks())
    # pt layout: strip j occupies pt_all[:, strip_base[j] : +T-128j] contiguous
    strip_base = {}
    strip_end_win = {}
    for (j, t0, w, fill) in all_chunks:
        if j not in strip_base:
            strip_base[j] = fill
        strip_end_win[j] = fill // WIN  # last window touching strip j

    win_tiles = {}
    pending = []              # chunks of the newest un-exped window
    next_block = 0            # next output block group to emit
    next_vtr = 0              # next v block to transpose
    qtiles = {}

    def emit_vtr(j):
        # v block j via PE transpose (XBAR DMA transposes cost ~1.2us each
        # on a DGE queue -- way too slow)
        tr = ps.tile([P, H], BF16, tag="acc", bufs=2, name=f"vtr{j}")
        nc.tensor.transpose(tr[:], vTsb[:, ds(P * j, P)], ident[0:H, 0:H])
        nc.vector.tensor_copy(v_sb[:, j, 0:H], tr[:])

    def emit_norm(i, src_psum):
        # transpose block i back to [t, h] (PE), then normalize on DVE
        if i % 4 == 0:
            qtiles[i // 4] = (
                sb.tile([P, 4, H], F32, tag="outf", bufs=2, name=f"outf{i}"),
                sb.tile([P, 4], F32, tag="recip", bufs=2, name=f"recip{i}"),
            )
        outf, r = qtiles[i // 4]
        b = i % 4
        tr = ps.tile([P, H + 1], BF16, tag="acc", bufs=2, name=f"otr{i}")
        nc.tensor.transpose(
            tr[:], outd[:, ts(i, P)], ident[0:H + 1, 0:H + 1]
        )
        nc.vector.reciprocal(r[:, ds(b, 1)], tr[:, H:H + 1])
        nc.vector.tensor_scalar_mul(outf[:, b, :], tr[:, 0:H], r[:, ds(b, 1)])
        if b == 3:
            nc.sync.dma_start(out_v[i // 4], outf[:])

    def emit_block(i):
        # output block i: consecutive accumulation group over strips j<=i
        # (PSUM accumulation is only valid in uninterrupted start..stop
        # groups on hardware)
        oa = ps.tile([P, P], F32, tag="acc", bufs=2, name=f"oacc{i}")
        for j in range(i + 1):
            nc.tensor.matmul(
                oa[0:H + 1, :],
                v_sb[:, j, 0:H + 1],
                pt_all[:, ds(strip_base[j] + P * (i - j), P)],
                start=(j == 0), stop=(j == i),
            )
        nc.vector.tensor_copy(outd[0:H + 1, ts(i, P)], oa[0:H + 1, :])
        emit_norm(i, oa)

    def emit_q3_wide(iv):
        # blocks 12-15 as one wide N=512 accumulation group (16 MMs instead
        # of 58): out cols [1536, 2048), strips 13-15 enter partial-width
        oa = ps.tile([P, 512], F32, tag="acc", bufs=2, name="oaccW")
        for j in range(NT):
            lo = max(1536, P * j)
            nc.tensor.matmul(
                oa[0:H + 1, ds(lo - 1536, 2048 - lo)],
                v_sb[:, j, 0:H + 1],
                pt_all[:, ds(strip_base[j] + lo - P * j, 2048 - lo)],
                start=(j == 0), stop=(j == NT - 1),
                skip_group_check=True,
            )
        nc.vector.tensor_copy(outd[0:H + 1, 1536:2048], oa[0:H + 1, :])
        for i in range(12, NT):
            emit_norm(i, None)

    def flush(wid):
        # exp the filled window; then (while ACT runs) masks, v transposes,
        # and any output-block groups whose strips are all exp'd
        nonlocal pending, next_block, next_vtr
        if not pending:
            return
        wt, fill = win_tiles.pop(wid)
        pt0 = wid * WIN
        nc.scalar.activation(pt_all[:, ds(pt0, fill)], wt[:, 0:fill], Exp,
                             scale=SCALE)
        for (j, t0, w, fpos) in pending:
            pt_off = pt0 + fpos
            # mask any part of this chunk inside the strip's diagonal block
            dlo, dhi = P * j, P * j + P
            mlo, mhi = max(t0, dlo), min(t0 + w, dhi)
            if mlo < mhi:
                nc.gpsimd.tensor_mul(
                    pt_all[:, ds(pt_off + (mlo - t0), mhi - mlo)],
                    pt_all[:, ds(pt_off + (mlo - t0), mhi - mlo)],
                    mask_sb[:, ds(mlo - dlo, mhi - mlo)],
                )
        pending = []
        if wid == 0:
            emit_proj(3)
        while next_vtr < NT and next_vtr < 4 * (wid + 1):
            emit_vtr(next_vtr)
            next_vtr += 1
        while next_block < 12 and strip_end_win[next_block] < wid:
            emit_block(next_block)
            next_block += 1

    # first window needs only proj t4 0,1 (proj2 next, proj3 at flush 0)
    emit_proj(0)
    emit_proj(1)
    emit_proj(2)
    cur_wid = 0
    for (j, t0, w, fill) in all_chunks:
        wid, fpos = fill // WIN, fill % WIN
        if wid != cur_wid:
            flush(cur_wid)
            cur_wid = wid
        if fpos == 0:
            wt = ps.tile([P, WIN], F32, tag="win", bufs=3, name=f"win{wid}")
            win_tiles[wid] = (wt, 0)
        wt, wfill = win_tiles[wid]
        assert wfill == fpos, (wfill, fpos)
        rg = (fill // 512) % 2
        stat = kA if rg == 0 else kB
        mov = qA if rg == 0 else qB
        nc.tensor.matmul(
            wt[:, ds(fpos, w)],
            stat[:, ds(P * j, P)],
            mov[:, ds(t0, w)],
            start=True, stop=True,
        )
        win_tiles[wid] = (wt, wfill + w)
        pending.append((j, t0, w, fpos))
    flush(cur_wid)
    # blocks 10-11 unlock only at the final flush; 12-15 via the wide group
    while next_block < 12:
        emit_block(next_block)
        next_block += 1
    emit_q3_wide(None)
    if dbg_d is not None:
        nc.sync.dma_start(dbg_d[:, 0:NT * (H + 1)],
                          v_sb.rearrange("p j h -> p (j h)"))
        nc.sync.dma_start(dbg_d[:, 2048:2048 + 4096],
                          pt_all[:, 0:4096])


def _build_program(num_devices=B, debug_out=False):
    nc = bacc.Bacc("TRN2", target_bir_lowering=False, debug=False,
                   num_devices=num_devices)
    xT_d = nc.dram_tensor("xT", [NCC, 4, P, 512], BF16,
                          kind="ExternalInput").ap()
    wqk_d = nc.dram_tensor("wqk", [P, NCC, P], BF16, kind="ExternalInput").ap()
    wv_d = nc.dram_tensor("wv", [P, NCC, H], BF16, kind="ExternalInput").ap()
    mask_d = nc.dram_tensor("mask", [P, P], BF16, kind="ExternalInput").ap()
    ident_d = nc.dram_tensor("ident", [P, P], BF16, kind="ExternalInput").ap()
    out_d = nc.dram_tensor("out", [T, H], F32, kind="ExternalOutput").ap()
    dbg_d = None
    if debug_out:
        dbg_d = nc.dram_tensor("dbg", [P, 8192], BF16,
                               kind="ExternalOutput").ap()
    from contextlib import ExitStack

    with tile.TileContext(nc) as tc:
        with ExitStack() as ctx:
            _emit(tc, xT_d, wqk_d, wv_d, mask_d, ident_d, out_d, ctx,
                  dbg_d=dbg_d)
    nc.compile()
    return nc


def _host_inputs(x, Wq, Wk, Wv):
    bf = ml_dtypes.bfloat16
    xT = np.ascontiguousarray(np.transpose(x, (0, 2, 1))).astype(bf)
    # chunk-contiguous DRAM layout [c-chunk, t4-chunk, 128, 512]
    Bn = x.shape[0]
    xT = np.ascontiguousarray(
        xT.reshape(Bn, NCC, P, 4, 512).transpose(0, 1, 3, 2, 4)
    )
    # wqk[p, c, 0:64] = Wq[c*128+p, :], wqk[p, c, 64:128] = Wk[c*128+p, :]
    wqk = np.concatenate([Wq, Wk], axis=1).reshape(NCC, P, 2 * H)
    wqk = np.ascontiguousarray(np.transpose(wqk, (1, 0, 2))).astype(bf)
    wv = np.ascontiguousarray(
        np.transpose(Wv.reshape(NCC, P, H), (1, 0, 2))
    ).astype(bf)
    # mask[s, t] = 1 where s <= t (transposed-causal diagonal block)
    mask = np.triu(np.ones((P, P), dtype=np.float32)).astype(bf)
    identity = np.eye(P, dtype=np.float32).astype(bf)
    return xT, wqk, wv, mask, identity


def kernel(x, Wq, Wk, Wv):
    global LAST_RESULT, _PROGRAM
    assert x.shape == (B, T, C), x.shape
    if _PROGRAM is None:
        _PROGRAM = _build_program()
    nc = _PROGRAM

    xT, wqk, wv, mask, identity = _host_inputs(x, Wq, Wk, Wv)
    in_maps = [
        {"xT": xT[b], "wqk": wqk, "wv": wv, "mask": mask, "ident": identity}
        for b in range(B)
    ]
    trace = bool(int(os.environ.get("KERNEL_TRACE", "0")))
    kw = {}
    td = os.environ.get("KERNEL_TRACE_DIR")
    if td:
        kw["tmpdir"] = td
    LAST_RESULT = run_bass_kernel_spmd(
        nc, in_maps, list(range(B)), trace=trace, **kw
    )
    out = np.stack([LAST_RESULT.results[b]["out"] for b in range(B)], axis=0)
    return out.astype(np.float32)
